# revision 1
# baseline (speedup 1.0000x reference)
"""Two-layer GAT (GATConv 128->64x4 concat, relu, GATConv 256->2) on 8 TRN2
NeuronCores, self-contained.

Sharding: edges are partitioned by destination node. Each core owns a
contiguous slice of 6250 destination nodes (49 windows of 128 nodes). Within a
window, edges are tiled in groups of 128; segment softmax + weighted
aggregation are computed with one-hot matmuls accumulating into PSUM.

Device pipeline per core:
  Phase A: h-table[n] = [x@W1 (256 f32) | al_src (4) | al_dst (4) | pad]
           for ALL nodes (replicated compute), written to a private HBM table.
  Phase B: per window: dma_gather of h-table rows by edge src (int16 indices
           against a base biased by +32768 so 50k rows are addressable),
           e = al_s[src] + al_d[dst] (dst side via transpose(one-hot) matmul),
           p = exp(leaky_relu(e)), out = onehot^T @ [h[src]*p | p] in PSUM,
           out1 = num/denom + b1, relu, h2lite = relu1 @ [W2|W2 a_s2|W2 a_d2].
  AllGather h2lite slices -> full table.
  Phase C: same window structure for layer 2, output [6272, 2] rows per core.
"""

import os
import sys
import time

sys.path.insert(0, "/opt/trn_rl_repo")

import numpy as np

import concourse.bacc as bacc
import concourse.mybir as mybir
import concourse.tile as tile
from concourse.bass_utils import run_bass_kernel_spmd
from concourse.library_config import mlp
from concourse.masks import make_identity

# problem constants (hardcoded per harness contract)
N = 50000
INCH = 128
HID = 64
HEADS = 4
OUT = 2
NEG = 0.2
CORES = 8
NPC = N // CORES          # 6250 dst nodes per core
P = 128
W = 49                    # windows of 128 dst nodes per core (49*128 = 6272)
NPCP = W * P              # padded nodes per core (6272)
NROW1 = 392 * P           # l1 table rows (50176 >= N, >= 7*NPC + NPCP)
NROW2 = CORES * NPCP      # h2lite table rows (50176)
BIAS = 32768              # int16 gather index bias
EPS = 1e-16

f32 = mybir.dt.float32
i16 = mybir.dt.int16
i32 = mybir.dt.int32

LAST_EXEC_NS = None
_cache = {}


def _wrap_idx_stream(arr):
    """arr [W, C] int16 -> [128, W*C//16]: per-window 16-partition wrap,
    replicated to all 8 Q7 groups."""
    Wn, C = arr.shape
    a = arr.reshape(Wn, C // 16, 16).transpose(2, 0, 1).reshape(16, Wn * (C // 16))
    return np.tile(a, (8, 1)).copy()


def _chunks(K):
    """[(tile_off, ntiles)] with ntiles <= 8 (1024-idx dma_gather limit)."""
    out = []
    off = 0
    while off < K:
        n = min(8, K - off)
        out.append((off, n))
        off += n
    return out


def _build(K):
    C = K * P
    phases = os.environ.get("KPHASES", "ABGC")
    nc = bacc.Bacc("TRN2", target_bir_lowering=False, debug=False, num_devices=CORES)

    # inputs
    x_d = nc.dram_tensor("x", [NROW1, INCH], f32, kind="ExternalInput")
    wcat_d = nc.dram_tensor("wcat", [INCH, 264], f32, kind="ExternalInput")
    w2cat_d = nc.dram_tensor("w2cat", [P, 8], f32, kind="ExternalInput")
    b1_d = nc.dram_tensor("b1", [1, 256], f32, kind="ExternalInput")
    b2_d = nc.dram_tensor("b2", [1, 2], f32, kind="ExternalInput")
    idx1_d = nc.dram_tensor("idx1", [P, W * C // 16], i16, kind="ExternalInput")
    idx2_d = nc.dram_tensor("idx2", [P, W * C // 16], i16, kind="ExternalInput")
    idxd1_d = nc.dram_tensor("idxd1", [P, W * C // 16], i16, kind="ExternalInput")
    idxd2_d = nc.dram_tensor("idxd2", [P, W * C // 16], i16, kind="ExternalInput")
    slots_d = nc.dram_tensor("slots", [P, W * K], f32, kind="ExternalInput")

    out_d = nc.dram_tensor("out", [NPCP, OUT], f32, kind="ExternalOutput")
    dbg_o = nc.dram_tensor("dbg", [P, 264], f32, kind="ExternalOutput")
    dbg2_o = nc.dram_tensor("dbg2", [P, 4], f32, kind="ExternalOutput")

    # scratch
    tab = nc.dram_tensor("tab", [NROW1, 320], f32)
    altab = nc.dram_tensor("altab", [NROW1, 64], f32)
    h2own = nc.dram_tensor("h2own", [NPCP, 64], f32)
    h2full = nc.dram_tensor("h2full", [NROW2, 64], f32, addr_space="Shared")

    LR = mybir.AluOpType
    AF = mybir.ActivationFunctionType

    with tile.TileContext(nc) as tc:
        with tc.tile_pool(name="const", bufs=1) as cpool:
            nc.gpsimd.load_library(mlp)

            ident = cpool.tile([P, P], f32)
            make_identity(nc, ident[:])
            iota_i = cpool.tile([P, P], i32)
            nc.gpsimd.iota(iota_i[:], pattern=[[1, P]], base=0, channel_multiplier=0)
            iota_f = cpool.tile([P, P], f32)
            nc.vector.tensor_copy(iota_f[:], iota_i[:])
            ones = cpool.tile([1, P], f32)
            nc.vector.memset(ones[:], 1.0)

            wcat_sb = cpool.tile([INCH, 264], f32)
            nc.sync.dma_start(out=wcat_sb[:], in_=wcat_d[:, :])
            w2cat_sb = cpool.tile([P, 8], f32)
            nc.sync.dma_start(out=w2cat_sb[:], in_=w2cat_d[:, :])
            b1row = cpool.tile([1, 256], f32)
            nc.sync.dma_start(out=b1row[:], in_=b1_d[:, :])
            b2row = cpool.tile([1, 2], f32)
            nc.sync.dma_start(out=b2row[:], in_=b2_d[:, :])
            idx1_sb = cpool.tile([P, W * C // 16], i16)
            nc.sync.dma_start(out=idx1_sb[:], in_=idx1_d[:, :])
            idx2_sb = cpool.tile([P, W * C // 16], i16)
            nc.sync.dma_start(out=idx2_sb[:], in_=idx2_d[:, :])
            idxd1_sb = cpool.tile([P, W * C // 16], i16)
            nc.sync.dma_start(out=idxd1_sb[:], in_=idxd1_d[:, :])
            idxd2_sb = cpool.tile([P, W * C // 16], i16)
            nc.sync.dma_start(out=idxd2_sb[:], in_=idxd2_d[:, :])
            slots_sb = cpool.tile([P, W * K], f32)
            nc.sync.dma_start(out=slots_sb[:], in_=slots_d[:, :])

            # replicated biases
            with tc.tile_pool(name="psum_b", bufs=1, space="PSUM") as psb:
                b1_ps = psb.tile([P, 256], f32, space="PSUM")
                nc.tensor.matmul(out=b1_ps[:], lhsT=ones[:], rhs=b1row[:], start=True, stop=True)
                b1_rep = cpool.tile([P, 256], f32)
                nc.scalar.copy(b1_rep[:], b1_ps[:])
                b2_ps = psb.tile([P, 2], f32, space="PSUM")
                nc.tensor.matmul(out=b2_ps[:], lhsT=ones[:], rhs=b2row[:], start=True, stop=True)
                b2_rep = cpool.tile([P, 2], f32)
                nc.scalar.copy(b2_rep[:], b2_ps[:])

            reps = int(os.environ.get("KREPS", "1"))
            for _rep in range(reps):
              # ---------------- Phase A: node features table ----------------
              if "A" in phases:
                  with (
                      tc.tile_pool(name="sbufA", bufs=int(os.environ.get("ABUFS", "4"))) as pa,
                      tc.tile_pool(name="psumA", bufs=int(os.environ.get("APBUFS", "4")), space="PSUM") as ppa,
                  ):
                      for i in range(NROW1 // P):
                          xt = pa.tile([P, INCH], f32)
                          nc.sync.dma_start(out=xt[:], in_=x_d[i * P : (i + 1) * P, :])
                          xT_ps = ppa.tile([P, P], f32, space="PSUM")
                          nc.tensor.transpose(out=xT_ps[:], in_=xt[:], identity=ident[:])
                          xT = pa.tile([P, P], f32)
                          nc.scalar.copy(xT[:], xT_ps[:])
                          h_ps = ppa.tile([P, 264], f32, space="PSUM")
                          nc.tensor.matmul(out=h_ps[:], lhsT=xT[:], rhs=wcat_sb[:], start=True, stop=True)
                          stg = pa.tile([P, 264], f32)
                          nc.scalar.copy(stg[:], h_ps[:])
                          nc.sync.dma_start(
                              out=tab[i * P : (i + 1) * P, 0:264], in_=stg[:]
                          )
                          nc.sync.dma_start(
                              out=altab[i * P : (i + 1) * P, 0:8], in_=stg[:, 256:264]
                          )
                  # debug: dump one table tile
                  with tc.tile_pool(name="dbgp", bufs=1) as dp:
                      dt_ = dp.tile([P, 264], f32)
                      nc.sync.dma_start(out=dt_[:], in_=tab[0:P, 0:264])
                      nc.sync.dma_start(out=dbg_o[:, :], in_=dt_[:])

              # ---------------- Phase B: layer-1 edge aggregation ----------------
              if "B" in phases:
                  with (
                      tc.tile_pool(name="sbufB", bufs=int(os.environ.get("GBUFS", "2"))) as pb,
                      tc.tile_pool(name="sbufBs", bufs=int(os.environ.get("BUFS", "6"))) as pbs,
                      tc.tile_pool(name="psumAgg", bufs=2, space="PSUM") as pagg,
                      tc.tile_pool(name="psumT", bufs=2, space="PSUM") as pt,
                      tc.tile_pool(name="psumE", bufs=int(os.environ.get("PEBUFS", "1")), space="PSUM") as pe,
                      tc.tile_pool(name="psumH", bufs=1, space="PSUM") as ph,
                  ):
                      for w in range(W):
                          gbuf = pb.tile([P, K, 320], f32, tag="gbuf")
                          for (toff, ntl) in _chunks(K):
                              nc.gpsimd.dma_gather(
                                  gbuf[:, toff : toff + ntl, :],
                                  tab[BIAS:, :],
                                  idx1_sb[:, w * (C // 16) + toff * 8 : w * (C // 16) + (toff + ntl) * 8],
                                  ntl * P,
                                  ntl * P,
                                  320,
                                  queue_num=0,
                              )
                          gbufd = pb.tile([P, K, 64], f32, tag="gbufd")
                          for (toff, ntl) in _chunks(K):
                              nc.gpsimd.dma_gather(
                                  gbufd[:, toff : toff + ntl, :],
                                  altab[BIAS:, :],
                                  idxd1_sb[:, w * (C // 16) + toff * 8 : w * (C // 16) + (toff + ntl) * 8],
                                  ntl * P,
                                  ntl * P,
                                  64,
                                  queue_num=0,
                              )
                          agg_ps = pagg.tile([P, 260], f32, space="PSUM", tag="agg")
                          for k in range(K):
                              onehot = pbs.tile([P, P], f32, tag="onehot")
                              nc.vector.tensor_scalar(
                                  out=onehot[:],
                                  in0=iota_f[:],
                                  scalar1=slots_sb[:, w * K + k : w * K + k + 1],
                                  scalar2=None,
                                  op0=LR.is_equal,
                              )
                              e_sb = pbs.tile([P, 4], f32, tag="e")
                              nc.vector.tensor_tensor(
                                  out=e_sb[:], in0=gbuf[:, k, 256:260],
                                  in1=gbufd[:, k, 4:8], op=LR.add,
                              )
                              lr_sb = pbs.tile([P, 4], f32, tag="lr")
                              nc.vector.scalar_tensor_tensor(
                                  out=lr_sb[:], in0=e_sb[:], scalar=NEG, in1=e_sb[:],
                                  op0=LR.mult, op1=LR.max,
                              )
                              p_sb = pbs.tile([P, 4], f32, tag="p")
                              nc.scalar.activation(p_sb[:], lr_sb[:], AF.Exp)
                              msg = pbs.tile([P, 260], f32, tag="msg")
                              for h in range(HEADS):
                                  if h < int(os.environ.get("HACT", "1")):
                                      nc.scalar.mul(
                                          msg[:, h * HID : (h + 1) * HID],
                                          gbuf[:, k, h * HID : (h + 1) * HID],
                                          p_sb[:, h : h + 1],
                                      )
                                  else:
                                      nc.vector.tensor_scalar_mul(
                                          msg[:, h * HID : (h + 1) * HID],
                                          gbuf[:, k, h * HID : (h + 1) * HID],
                                          p_sb[:, h : h + 1],
                                      )
                              nc.vector.tensor_copy(msg[:, 256:260], p_sb[:])
                              nc.tensor.matmul(
                                  out=agg_ps[:], lhsT=onehot[:], rhs=msg[:],
                                  start=(k == 0), stop=(k == K - 1),
                              )
                          # window readout
                          den = pbs.tile([P, 4], f32, tag="den")
                          nc.vector.tensor_scalar(
                              out=den[:], in0=agg_ps[:, 256:260], scalar1=EPS,
                              scalar2=None, op0=LR.add,
                          )
                          rec = pbs.tile([P, 4], f32, tag="rec")
                          nc.vector.reciprocal(rec[:], den[:])
                          relu1 = pbs.tile([P, 256], f32, tag="relu1")
                          for h in range(HEADS):
                              nc.scalar.mul(
                                  relu1[:, h * HID : (h + 1) * HID],
                                  agg_ps[:, h * HID : (h + 1) * HID],
                                  rec[:, h : h + 1],
                              )
                          nc.vector.tensor_tensor(
                              out=relu1[:], in0=relu1[:], in1=b1_rep[:], op=LR.add
                          )
                          nc.scalar.activation(relu1[:], relu1[:], AF.Relu)
                          h2_ps = ph.tile([P, 4], f32, space="PSUM", tag="h2")
                          for half in range(2):
                              rT_ps = pt.tile([P, P], f32, space="PSUM", tag="ohT")
                              nc.tensor.transpose(
                                  out=rT_ps[:], in_=relu1[:, half * P : (half + 1) * P],
                                  identity=ident[:],
                              )
                              rT = pbs.tile([P, P], f32, tag="ohTs")
                              nc.scalar.copy(rT[:], rT_ps[:])
                              nc.tensor.matmul(
                                  out=h2_ps[:], lhsT=rT[:],
                                  rhs=w2cat_sb[:, half * 4 : (half + 1) * 4],
                                  start=(half == 0), stop=(half == 1),
                              )
                          h2st = pbs.tile([P, 4], f32, tag="h2st")
                          nc.vector.tensor_copy(h2st[:], h2_ps[:])
                          nc.sync.dma_start(
                              out=h2own[w * P : (w + 1) * P, 0:4], in_=h2st[:]
                          )
                  # debug: dump first h2own tile
                  with tc.tile_pool(name="dbgp2", bufs=1) as dp2:
                      dt2 = dp2.tile([P, 4], f32)
                      nc.sync.dma_start(out=dt2[:], in_=h2own[0:P, 0:4])
                      nc.sync.dma_start(out=dbg2_o[:, :], in_=dt2[:])

              # ---------------- AllGather h2lite ----------------
              if "G" in phases:
                  nc.gpsimd.collective_compute(
                      "AllGather",
                      mybir.AluOpType.bypass,
                      replica_groups=[list(range(CORES))],
                      ins=[h2own.ap().opt()],
                      outs=[h2full.ap().opt()],
                  )

              # ---------------- Phase C: layer-2 edge aggregation ----------------
              if "C" in phases:
                  with (
                      tc.tile_pool(name="sbufC", bufs=int(os.environ.get("GBUFS", "2"))) as pc,
                      tc.tile_pool(name="sbufCs", bufs=int(os.environ.get("BUFS", "6"))) as pcs,
                      tc.tile_pool(name="psumAgg2", bufs=2, space="PSUM") as pagg2,
                      tc.tile_pool(name="psumT2", bufs=2, space="PSUM") as pt2,
                      tc.tile_pool(name="psumE2", bufs=int(os.environ.get("PEBUFS", "1")), space="PSUM") as pe2,
                  ):
                      for w in range(W):
                          g2 = pc.tile([P, K, 64], f32, tag="g2")
                          for (toff, ntl) in _chunks(K):
                              nc.gpsimd.dma_gather(
                                  g2[:, toff : toff + ntl, :],
                                  h2full[BIAS:, :],
                                  idx2_sb[:, w * (C // 16) + toff * 8 : w * (C // 16) + (toff + ntl) * 8],
                                  ntl * P,
                                  ntl * P,
                                  64,
                                  queue_num=0,
                              )
                          g2d = pc.tile([P, K, 64], f32, tag="g2d")
                          for (toff, ntl) in _chunks(K):
                              nc.gpsimd.dma_gather(
                                  g2d[:, toff : toff + ntl, :],
                                  h2full[BIAS:, :],
                                  idxd2_sb[:, w * (C // 16) + toff * 8 : w * (C // 16) + (toff + ntl) * 8],
                                  ntl * P,
                                  ntl * P,
                                  64,
                                  queue_num=0,
                              )
                          agg2_ps = pagg2.tile([P, 3], f32, space="PSUM", tag="agg2")
                          for k in range(K):
                              onehot = pcs.tile([P, P], f32, tag="onehot2")
                              nc.vector.tensor_scalar(
                                  out=onehot[:],
                                  in0=iota_f[:],
                                  scalar1=slots_sb[:, w * K + k : w * K + k + 1],
                                  scalar2=None,
                                  op0=LR.is_equal,
                              )
                              e_sb = pcs.tile([P, 1], f32, tag="e2")
                              nc.vector.tensor_tensor(
                                  out=e_sb[:], in0=g2[:, k, 2:3],
                                  in1=g2d[:, k, 3:4], op=LR.add
                              )
                              lr_sb = pcs.tile([P, 1], f32, tag="lr2")
                              nc.vector.scalar_tensor_tensor(
                                  out=lr_sb[:], in0=e_sb[:], scalar=NEG, in1=e_sb[:],
                                  op0=LR.mult, op1=LR.max,
                              )
                              p_sb = pcs.tile([P, 1], f32, tag="p2")
                              nc.scalar.activation(p_sb[:], lr_sb[:], AF.Exp)
                              msg = pcs.tile([P, 3], f32, tag="msg2")
                              nc.scalar.mul(msg[:, 0:2], g2[:, k, 0:2], p_sb[:, 0:1])
                              nc.vector.tensor_copy(msg[:, 2:3], p_sb[:])
                              nc.tensor.matmul(
                                  out=agg2_ps[:], lhsT=onehot[:], rhs=msg[:],
                                  start=(k == 0), stop=(k == K - 1),
                              )
                          den = pcs.tile([P, 1], f32, tag="den2")
                          nc.vector.tensor_scalar(
                              out=den[:], in0=agg2_ps[:, 2:3], scalar1=EPS,
                              scalar2=None, op0=LR.add,
                          )
                          rec = pcs.tile([P, 1], f32, tag="rec2")
                          nc.vector.reciprocal(rec[:], den[:])
                          o2 = pcs.tile([P, OUT], f32, tag="o2")
                          nc.scalar.mul(o2[:], agg2_ps[:, 0:2], rec[:, 0:1])
                          nc.vector.tensor_tensor(out=o2[:], in0=o2[:], in1=b2_rep[:], op=LR.add)
                          nc.sync.dma_start(out=out_d[w * P : (w + 1) * P, :], in_=o2[:])

    nc.compile()
    return nc


def _preprocess(x, edge_index, W1, a_src1, a_dst1, b1, W2, a_src2, a_dst2, b2):
    src = np.concatenate([np.asarray(edge_index[0]), np.arange(N)]).astype(np.int64)
    dst = np.concatenate([np.asarray(edge_index[1]), np.arange(N)]).astype(np.int64)

    core = dst // NPC
    loc = dst - core * NPC
    win = loc >> 7
    slot = loc & 127
    gid = core * W + win
    order = np.argsort(gid, kind="stable")
    counts = np.bincount(gid, minlength=CORES * W)
    K = int(np.ceil((counts.max() + 1) / P))
    while counts.max() > K * P - len(_chunks(K)):
        K += 1
    C = K * P

    starts = np.zeros(CORES * W, np.int64)
    starts[1:] = np.cumsum(counts)[:-1]
    within = np.arange(len(order)) - starts[gid[order]]
    # map within-window rank -> position, skipping the reserved last slot of
    # each gather chunk (keeps every chunk's final index non-negative)
    usable = np.array([n * P - 1 for (_, n) in _chunks(K)], np.int64)
    cumu = np.cumsum(usable)
    ci = np.searchsorted(cumu, within, side="right")
    pos = gid[order] * C + within + ci

    s_sorted = src[order]
    idx1 = np.full(CORES * W * C, BIAS, np.int64)
    idx1[pos] = s_sorted
    idx2 = np.full(CORES * W * C, BIAS, np.int64)
    idx2[pos] = (s_sorted // NPC) * NPCP + (s_sorted % NPC)
    d_sorted = dst[order]
    idxd1 = np.full(CORES * W * C, BIAS, np.int64)
    idxd1[pos] = d_sorted
    idxd2 = np.full(CORES * W * C, BIAS, np.int64)
    idxd2[pos] = (d_sorted // NPC) * NPCP + (d_sorted % NPC)
    slotv = np.full(CORES * W * C, float(P), np.float32)
    slotv[pos] = slot[order].astype(np.float32)

    idx1 = (idx1 - BIAS).astype(np.int16).reshape(CORES, W, C)
    idx2 = (idx2 - BIAS).astype(np.int16).reshape(CORES, W, C)
    idxd1 = (idxd1 - BIAS).astype(np.int16).reshape(CORES, W, C)
    idxd2 = (idxd2 - BIAS).astype(np.int16).reshape(CORES, W, C)
    slotv = slotv.reshape(CORES, W, K, P)


    # weights
    W1 = np.asarray(W1, np.float32)
    W1r = W1.reshape(INCH, HEADS, HID)
    wa_s = np.einsum("ihc,hc->ih", W1r, np.asarray(a_src1, np.float32))
    wa_d = np.einsum("ihc,hc->ih", W1r, np.asarray(a_dst1, np.float32))
    wcat = np.concatenate([W1, wa_s, wa_d], axis=1).astype(np.float32)

    W2 = np.asarray(W2, np.float32)
    w2s = W2 @ np.asarray(a_src2, np.float32)[0]
    w2d = W2 @ np.asarray(a_dst2, np.float32)[0]
    w2cat = np.concatenate([W2, w2s[:, None], w2d[:, None]], axis=1).astype(np.float32)
    w2cat = np.concatenate([w2cat[:P], w2cat[P:]], axis=1)  # [128, 8]

    x_pad = np.zeros((NROW1, INCH), np.float32)
    x_pad[:N] = np.asarray(x, np.float32)

    in_maps = []
    for c in range(CORES):
        in_maps.append(
            {
                "x": x_pad,
                "wcat": wcat,
                "w2cat": w2cat,
                "b1": np.asarray(b1, np.float32).reshape(1, 256),
                "b2": np.asarray(b2, np.float32).reshape(1, 2),
                "idx1": _wrap_idx_stream(idx1[c]),
                "idx2": _wrap_idx_stream(idx2[c]),
                "idxd1": _wrap_idx_stream(idxd1[c]),
                "idxd2": _wrap_idx_stream(idxd2[c]),
                "slots": slotv[c].transpose(2, 0, 1).reshape(P, W * K).copy(),
            }
        )
    return K, in_maps


class _Runner:
    """Persistent compiled runner: jit once, device-resident inputs, so
    repeated calls time only execution (+ dispatch)."""

    def __init__(self, nc):
        import jax
        from jax.sharding import Mesh, PartitionSpec, NamedSharding
        from jax.experimental.shard_map import shard_map
        from concourse import bass2jax
        import concourse.mybir as mb

        bass2jax.install_neuronx_cc_hook()
        self.jax = jax
        self.nc = nc
        part_name = nc.partition_id_tensor.name if nc.partition_id_tensor else None
        in_names, out_names, out_avals, zero_outs = [], [], [], []
        for alloc in nc.m.functions[0].allocations:
            if not isinstance(alloc, mb.MemoryLocationSet):
                continue
            name = alloc.memorylocations[0].name
            if alloc.kind == "ExternalInput":
                if name != part_name:
                    in_names.append(name)
            elif alloc.kind == "ExternalOutput":
                out_names.append(name)
                shape = tuple(alloc.tensor_shape)
                dtype = mb.dt.np(alloc.dtype)
                out_avals.append(jax.core.ShapedArray(shape, dtype))
                zero_outs.append(np.zeros(shape, dtype))
        self.in_names, self.out_names = in_names, out_names
        self.zero_outs = zero_outs
        n_params, n_outs = len(in_names), len(out_names)
        donate = tuple(range(n_params, n_params + n_outs))

        all_in_names = in_names + out_names + ([part_name] if part_name else [])

        def _body(*args):
            operands = list(args)
            if part_name is not None:
                operands.append(bass2jax.partition_id_tensor())
            outs = bass2jax._bass_exec_p.bind(
                *operands,
                out_avals=tuple(out_avals),
                in_names=tuple(all_in_names),
                out_names=tuple(out_names),
                lowering_input_output_aliases=(),
                sim_require_finite=True,
                sim_require_nnan=True,
                nc=nc,
            )
            return tuple(outs)

        devices = jax.devices()[:CORES]
        self.mesh = Mesh(np.asarray(devices), ("core",))
        self.spec = NamedSharding(self.mesh, PartitionSpec("core"))
        in_specs = (PartitionSpec("core"),) * (n_params + n_outs)
        out_specs = (PartitionSpec("core"),) * n_outs
        self.sharded = jax.jit(
            shard_map(_body, mesh=self.mesh, in_specs=in_specs,
                      out_specs=out_specs, check_rep=False),
            donate_argnums=donate, keep_unused=True,
        )
        self.dev_in = None

    def put_inputs(self, in_maps):
        self.dev_in = [
            self.jax.device_put(
                np.concatenate([np.asarray(m[n]) for m in in_maps], axis=0), self.spec
            )
            for n in self.in_names
        ]

    def execute(self):
        zeros = [
            self.jax.device_put(
                np.zeros((CORES * z.shape[0], *z.shape[1:]), z.dtype), self.spec
            )
            for z in self.zero_outs
        ]
        for z in zeros:
            z.block_until_ready()
        t0 = time.monotonic_ns()
        outs = self.sharded(*self.dev_in, *zeros)
        for o in outs:
            o.block_until_ready()
        dt = time.monotonic_ns() - t0
        res = [
            {
                name: np.asarray(outs[i]).reshape(CORES, *self.zero_outs[i].shape)[c]
                for i, name in enumerate(self.out_names)
            }
            for c in range(CORES)
        ]
        return res, dt


def run_on_device(in_maps, K):
    if K not in _cache:
        _cache[K] = _Runner(_build(K))
    runner = _cache[K]
    runner.put_inputs(in_maps)
    res, dt = runner.execute()
    global LAST_EXEC_NS
    LAST_EXEC_NS = dt
    return res


def kernel(x, edge_index, W1, a_src1, a_dst1, b1, W2, a_src2, a_dst2, b2):
    global LAST_EXEC_NS
    K, in_maps = _preprocess(
        x, edge_index, W1, a_src1, a_dst1, b1, W2, a_src2, a_dst2, b2
    )
    res = run_on_device(in_maps, K)
    out = np.concatenate([res[c]["out"][:NPC] for c in range(CORES)], axis=0)
    return out.astype(np.float32)



# revision 18
# speedup vs baseline: 111.5464x; 111.5464x over previous
"""Two-layer GAT (GATConv 128->64x4 concat, relu, GATConv 256->2) on 8 TRN2
NeuronCores, self-contained.

Layout: nodes globally sorted by in-degree (desc) and striped rank r ->
core r%8, local slot j=r//8 (window w=j//128, slot s=j%128). Every window is
degree-uniform, so the per-window tile count K_w = indeg(rank 1024w) is
near-minimal and identical across cores. Edges are stored rank-major: gather
stream position k*128+s holds the k-th in-edge of the window node at slot s,
so the dma_gather itself scatters edge payloads to their destination
partition - no one-hot matmuls. The self-loop of each node is forced to
k=0, so gbuf[:,0,260:264] is exactly al_dst of the window - no dst gather.

Per core:
  Phase A: tab[g] = [h=x@W1 (256 bf16) | al_src (4) | al_dst (4) | pad]
           for ALL 50304 permuted rows (replicated), 768B rows.
           Sentinel strips (rows c*6288+6272..+16) get al=-1e30.
  Phase B: per window: one gather of [128,K,384] bf16 by src; batched
           e=al_s+al_d, leaky-relu, exp (into gbuf), broadcast mul h*=p;
           K identity-matmuls accumulate [h*p | p] into PSUM; normalize,
           +b1, relu; h2lite = relu1 @ [W2|W2 a_s2|W2 a_d2] -> h2own row.
  AllGather h2own [6288,128]bf16 -> h2full [50304,128].
  Phase C: same window structure, 256B-row gathers, output [6272,2] f32.
"""

import os
import sys
import time

sys.path.insert(0, "/opt/trn_rl_repo")

import numpy as np

import concourse.bacc as bacc
import concourse.mybir as mybir
import concourse.tile as tile
from concourse.library_config import mlp
from concourse.masks import make_identity

# problem constants (hardcoded per harness contract)
N = 50000
INCH = 128
HID = 64
HEADS = 4
OUT = 2
NEG = 0.2
CORES = 8
P = 128
W = 49                     # windows per core
NPC = W * P                # 6272 real rows per core
NPCS = NPC + 16            # + sentinel strip = 6288
NROW = CORES * NPCS        # 50304 table rows (392 tiles + sentinel inside)
NTILE = NROW // P          # 393
BIAS = 32768               # int16 gather index bias
EPS = 1e-16
NEGBIG = -1e30

f32 = mybir.dt.float32
bf16 = mybir.dt.bfloat16
i16 = mybir.dt.int16

LAST_EXEC_NS = None
_cache = {}


def _wrap_idx_stream(arr):
    """arr [n] int16 -> [128, n//16]: 16-partition wrap, replicated to all
    8 Q7 groups."""
    n = arr.shape[0]
    a = arr.reshape(n // 16, 16).T
    return np.tile(a, (8, 1)).copy()


def _chunks(K):
    """[(tile_off, ntiles)] with ntiles <= 7 so ntiles*128+16 safe-column
    indices stay within the 1024-idx dma_gather limit."""
    out = []
    off = 0
    while off < K:
        n = min(7, K - off)
        out.append((off, n))
        off += n
    return out


def _wcols(K):
    """idx-stream columns for one window: per chunk ntl*8 data + 1 safe col."""
    return K * 8 + len(_chunks(K))


def _build(Ks):
    Ks = list(Ks)
    SK = sum(Ks)
    SKC = sum(_wcols(K) for K in Ks)
    coff_w = np.concatenate([[0], np.cumsum([_wcols(K) for K in Ks])]).astype(int)
    phases = os.environ.get("KPHASES", "ABGC")
    nc = bacc.Bacc("TRN2", target_bir_lowering=False, debug=False,
                   num_devices=CORES, num_swdge_queues=4)

    xt_d = nc.dram_tensor("xt", [P, NROW], bf16, kind="ExternalInput")
    wcat_d = nc.dram_tensor("wcat", [INCH, 264], bf16, kind="ExternalInput")
    w2cat_d = nc.dram_tensor("w2cat", [P, 2, 4], bf16, kind="ExternalInput")
    b1_d = nc.dram_tensor("b1", [1, 256], f32, kind="ExternalInput")
    b2_d = nc.dram_tensor("b2", [1, 2], f32, kind="ExternalInput")
    idx_d = nc.dram_tensor("idx", [P, SKC], i16, kind="ExternalInput")

    out_d = nc.dram_tensor("out", [NPC, OUT], f32, kind="ExternalOutput")

    tab = nc.dram_tensor("tab", [NROW, 384], bf16)
    h2own = nc.dram_tensor("h2own", [NPCS, 128], bf16)
    h2full = nc.dram_tensor("h2full", [NROW, 128], bf16, addr_space="Shared")

    LR = mybir.AluOpType
    AF = mybir.ActivationFunctionType

    with tile.TileContext(nc) as tc:
        with tc.tile_pool(name="const", bufs=1) as cpool:
            nc.gpsimd.load_library(mlp)

            ident = cpool.tile([P, P], bf16)
            make_identity(nc, ident[:])
            ones = cpool.tile([1, P], f32)
            nc.vector.memset(ones[:], 1.0)

            wcat_sb = cpool.tile([INCH, 264], bf16)
            nc.sync.dma_start(out=wcat_sb[:], in_=wcat_d[:, :])
            w2cat_sb = cpool.tile([P, 2, 4], bf16)
            nc.sync.dma_start(out=w2cat_sb[:], in_=w2cat_d[:, :, :])
            b1row = cpool.tile([1, 256], f32)
            nc.sync.dma_start(out=b1row[:], in_=b1_d[:, :])
            b2row = cpool.tile([1, 2], f32)
            nc.sync.dma_start(out=b2row[:], in_=b2_d[:, :])
            idx_sb = cpool.tile([P, SKC], i16)
            nc.sync.dma_start(out=idx_sb[:], in_=idx_d[:, :])

            sent = cpool.tile([P, 128], bf16)
            nc.vector.memset(sent[:], NEGBIG)

            # replicated biases
            with tc.tile_pool(name="psum_b", bufs=1, space="PSUM") as psb:
                b1_ps = psb.tile([P, 256], f32, space="PSUM")
                nc.tensor.matmul(out=b1_ps[:], lhsT=ones[:], rhs=b1row[:],
                                 start=True, stop=True)
                b1_rep = cpool.tile([P, 256], f32)
                nc.scalar.copy(b1_rep[:], b1_ps[:])
                b2_ps = psb.tile([P, 2], f32, space="PSUM")
                nc.tensor.matmul(out=b2_ps[:], lhsT=ones[:], rhs=b2row[:],
                                 start=True, stop=True)
                b2_rep = cpool.tile([P, 2], f32)
                nc.scalar.copy(b2_rep[:], b2_ps[:])

            oacc = cpool.tile([P, W, OUT], f32)

            # ---------------- Phase A: node feature table ----------------
            if "A" in phases:
                TB = 8  # tiles per batch
                with (
                    tc.tile_pool(name="sbufA", bufs=3) as pa,
                    tc.tile_pool(name="psumA", bufs=4, space="PSUM") as ppa,
                ):
                    for i0 in range(0, NTILE, TB):
                        nb = min(TB, NTILE - i0)
                        xtb = pa.tile([P, nb * P], bf16, tag="xtb")
                        nc.sync.dma_start(
                            out=xtb[:], in_=xt_d[:, i0 * P : (i0 + nb) * P]
                        )
                        stg = pa.tile([P, nb, 384], bf16, tag="stg")
                        for t in range(nb):
                            h_ps = ppa.tile([P, 264], f32, space="PSUM")
                            nc.tensor.matmul(
                                out=h_ps[:], lhsT=xtb[:, t * P : (t + 1) * P],
                                rhs=wcat_sb[:], start=True, stop=True,
                            )
                            nc.scalar.copy(stg[:, t, 0:264], h_ps[:])
                        nc.sync.dma_start(
                            out=tab[i0 * P : (i0 + nb) * P, :].rearrange(
                                "(t p) c -> p t c", p=P
                            )[:, :, 0:264],
                            in_=stg[:, :, 0:264],
                        )
                # sentinel strips: al cols (256:264) of rows c*NPCS+6272..+16
                for c in range(CORES):
                    nc.sync.dma_start(
                        out=tab[c * NPCS + NPC : c * NPCS + NPC + 16, 256:264],
                        in_=sent[0:16, 0:8],
                    )

            gq = [0]  # global gather-chunk counter (queue round-robin)

            # ---------------- Phase B: layer-1 edge aggregation ----------------
            if "B" in phases:
                BL = int(os.environ.get("KBL", "9"))  # bisect level
                KMAX = max(Ks)
                with (
                    tc.tile_pool(name="gbufB", bufs=4) as pb,
                    tc.tile_pool(name="sbufB", bufs=3) as pbs,
                    tc.tile_pool(name="psumAgg", bufs=2, space="PSUM") as pagg,
                    tc.tile_pool(name="psumT", bufs=2, space="PSUM") as pt,
                    tc.tile_pool(name="psumH", bufs=2, space="PSUM") as ph,
                ):
                    for w in range(int(os.environ.get("KWIN", W))):
                        K = Ks[w]
                        co = int(coff_w[w])
                        gbuf = pb.tile([P, K + 1, 384], bf16, tag="gbuf")
                        for toff, ntl in _chunks(K):
                            nc.gpsimd.dma_gather(
                                gbuf[:, toff : toff + ntl + 1, :],
                                tab[BIAS:, :],
                                idx_sb[:, co : co + ntl * 8 + 1],
                                ntl * P + 16,
                                ntl * P + 16,
                                384,
                                queue_num=gq[0] % 4,
                            )
                            gq[0] += 1
                            co += ntl * 8 + 1
                        if BL < 1:
                            continue
                        # e = al_s + al_d(self-loop row k=0), leaky, exp -> p
                        e_sb = pbs.tile([P, K, 4], f32, tag="e")
                        nc.vector.tensor_tensor(
                            out=e_sb[:], in0=gbuf[:, 0:K, 256:260],
                            in1=gbuf[:, 0:1, 260:264].broadcast_to([P, K, 4]),
                            op=LR.add,
                        )
                        nc.vector.scalar_tensor_tensor(
                            out=e_sb[:], in0=e_sb[:], scalar=NEG, in1=e_sb[:],
                            op0=LR.mult, op1=LR.max,
                        )
                        if BL >= 2:
                            nc.scalar.activation(gbuf[:, 0:K, 256:260], e_sb[:], AF.Exp)
                        if BL < 3:
                            continue
                        # msg = h * p (broadcast per head), in place
                        g4 = gbuf[:, 0:K, 0:256].rearrange(
                            "p k (h c) -> p k h c", h=HEADS
                        )
                        nc.vector.tensor_tensor(
                            out=g4, in0=g4,
                            in1=gbuf[:, 0:K, 256:260][:, :, :, None].broadcast_to(
                                [P, K, HEADS, HID]
                            ),
                            op=LR.mult,
                        )
                        if BL < 4:
                            continue
                        # aggregate [msg | p] over K tiles into PSUM
                        agg_ps = pagg.tile([P, 260], f32, space="PSUM", tag="agg")
                        for k in range(K):
                            nc.tensor.matmul(
                                out=agg_ps[:], lhsT=ident[:],
                                rhs=gbuf[:, k, 0:260],
                                start=(k == 0), stop=(k == K - 1),
                            )
                        if BL < 5:
                            continue
                        den = pbs.tile([P, 4], f32, tag="den")
                        nc.vector.tensor_scalar(
                            out=den[:], in0=agg_ps[:, 256:260], scalar1=EPS,
                            scalar2=None, op0=LR.add,
                        )
                        rec = pbs.tile([P, 4], f32, tag="rec")
                        nc.vector.reciprocal(rec[:], den[:])
                        tmp = pbs.tile([P, 256], f32, tag="tmp")
                        nc.vector.tensor_tensor(
                            out=tmp[:].rearrange("p (h c) -> p h c", h=HEADS),
                            in0=agg_ps[:, 0:256].rearrange("p (h c) -> p h c", h=HEADS),
                            in1=rec[:, :, None].broadcast_to([P, HEADS, HID]),
                            op=LR.mult,
                        )
                        nc.vector.tensor_tensor(
                            out=tmp[:], in0=tmp[:], in1=b1_rep[:], op=LR.add
                        )
                        relu1 = pbs.tile([P, 256], bf16, tag="relu1")
                        nc.scalar.activation(relu1[:], tmp[:], AF.Relu)
                        if BL < 6:
                            continue
                        h2_ps = ph.tile([P, 4], f32, space="PSUM", tag="h2")
                        for half in range(2):
                            rT_ps = pt.tile([P, P], bf16, space="PSUM", tag="rT")
                            nc.tensor.transpose(
                                out=rT_ps[:],
                                in_=relu1[:, half * P : (half + 1) * P],
                                identity=ident[:],
                            )
                            rT = pbs.tile([P, P], bf16, tag="rTs")
                            nc.scalar.copy(rT[:], rT_ps[:])
                            nc.tensor.matmul(
                                out=h2_ps[:], lhsT=rT[:], rhs=w2cat_sb[:, half, :],
                                start=(half == 0), stop=(half == 1),
                            )
                        h2st = pbs.tile([P, 4], bf16, tag="h2st")
                        nc.scalar.copy(h2st[:], h2_ps[:])
                        nc.sync.dma_start(
                            out=h2own[w * P : (w + 1) * P, 0:4], in_=h2st[:]
                        )
                    # own sentinel strip rows 6272..6288
                    nc.sync.dma_start(
                        out=h2own[NPC : NPC + 16, :], in_=sent[0:16, :]
                    )

            # ---------------- AllGather h2 ----------------
            if "G" in phases:
                nc.gpsimd.collective_compute(
                    "AllGather",
                    mybir.AluOpType.bypass,
                    replica_groups=[list(range(CORES))],
                    ins=[h2own.ap().opt()],
                    outs=[h2full.ap().opt()],
                )

            # ---------------- Phase C: layer-2 edge aggregation ----------------
            if "C" in phases:
                with (
                    tc.tile_pool(name="gbufC", bufs=4) as pc,
                    tc.tile_pool(name="sbufC", bufs=3) as pcs,
                    tc.tile_pool(name="psumAgg2", bufs=2, space="PSUM") as pagg2,
                ):
                    for w in range(W):
                        K = Ks[w]
                        co = int(coff_w[w])
                        g2 = pc.tile([P, K + 1, 128], bf16, tag="g2")
                        for toff, ntl in _chunks(K):
                            nc.gpsimd.dma_gather(
                                g2[:, toff : toff + ntl + 1, :],
                                h2full[BIAS:, :],
                                idx_sb[:, co : co + ntl * 8 + 1],
                                ntl * P + 16,
                                ntl * P + 16,
                                128,
                                queue_num=gq[0] % 4,
                            )
                            gq[0] += 1
                            co += ntl * 8 + 1
                        e2 = pcs.tile([P, K, 1], f32, tag="e2")
                        nc.vector.tensor_tensor(
                            out=e2[:], in0=g2[:, 0:K, 2:3],
                            in1=g2[:, 0:1, 3:4].broadcast_to([P, K, 1]),
                            op=LR.add,
                        )
                        nc.vector.scalar_tensor_tensor(
                            out=e2[:], in0=e2[:], scalar=NEG, in1=e2[:],
                            op0=LR.mult, op1=LR.max,
                        )
                        nc.scalar.activation(g2[:, 0:K, 2:3], e2[:], AF.Exp)
                        nc.vector.tensor_tensor(
                            out=g2[:, 0:K, 0:2], in0=g2[:, 0:K, 0:2],
                            in1=g2[:, 0:K, 2:3].broadcast_to([P, K, 2]),
                            op=LR.mult,
                        )
                        agg2_ps = pagg2.tile([P, 3], f32, space="PSUM", tag="agg2")
                        for k in range(K):
                            nc.tensor.matmul(
                                out=agg2_ps[:], lhsT=ident[:], rhs=g2[:, k, 0:3],
                                start=(k == 0), stop=(k == K - 1),
                            )
                        den2 = pcs.tile([P, 1], f32, tag="den2")
                        nc.vector.tensor_scalar(
                            out=den2[:], in0=agg2_ps[:, 2:3], scalar1=EPS,
                            scalar2=None, op0=LR.add,
                        )
                        rec2 = pcs.tile([P, 1], f32, tag="rec2")
                        nc.vector.reciprocal(rec2[:], den2[:])
                        nc.vector.tensor_scalar(
                            out=oacc[:, w, :], in0=agg2_ps[:, 0:2],
                            scalar1=rec2[:, 0:1], scalar2=None, op0=LR.mult,
                        )
                        nc.vector.tensor_tensor(
                            out=oacc[:, w, :], in0=oacc[:, w, :], in1=b2_rep[:],
                            op=LR.add,
                        )
                    nc.sync.dma_start(
                        out=out_d[:, :].rearrange("(w p) c -> p w c", p=P),
                        in_=oacc[:],
                    )

    nc.compile()
    return nc


def _preprocess(x, edge_index, W1, a_src1, a_dst1, b1, W2, a_src2, a_dst2, b2):
    src = np.concatenate([np.asarray(edge_index[0]), np.arange(N)]).astype(np.int64)
    dst = np.concatenate([np.asarray(edge_index[1]), np.arange(N)]).astype(np.int64)
    E2 = len(src)

    indeg = np.bincount(dst, minlength=N)
    order_nodes = np.argsort(-indeg, kind="stable")       # rank -> node
    rank_of = np.empty(N, np.int64)
    rank_of[order_nodes] = np.arange(N)
    indeg_sorted = indeg[order_nodes]

    # per-window tile count: max in-degree among ranks [1024w, 1024(w+1))
    Ks = [int(indeg_sorted[1024 * w]) for w in range(W)]
    SK = sum(Ks)
    woff = np.concatenate([[0], np.cumsum(Ks)]).astype(np.int64)

    # permuted table row for each node: rank r -> core r%8, local j=r//8
    node_g = (rank_of % CORES) * NPCS + (rank_of // CORES)

    # within-node edge order, self-loop (last E2-N..E2) first
    is_loop = np.zeros(E2, np.int64)
    is_loop[len(src) - N :] = 0
    not_loop = np.ones(E2, np.int64)
    not_loop[len(src) - N :] = 0
    eorder = np.lexsort((not_loop, dst))                  # group by dst, loop first
    d_sorted = dst[eorder]
    starts = np.zeros(N, np.int64)
    cnt = np.bincount(d_sorted, minlength=N)
    starts[1:] = np.cumsum(cnt)[:-1]
    k_rank = np.arange(E2) - starts[d_sorted]             # k index per edge

    r = rank_of[d_sorted]
    core = r % CORES
    j = r // CORES
    w_idx = j // P
    s_idx = j % P
    # stream position within the core: (woff[w] + k)*128 + s
    pos = (woff[w_idx] + k_rank) * P + s_idx

    # per-core index streams, prefilled with sentinel rows (core0 strip)
    idx = np.empty((CORES, SK * P), np.int64)
    idx[:] = NPC + (np.arange(SK * P) % 16)               # rows 6272..6287 of core0
    idx[core, pos] = node_g[src[eorder]]
    idx16 = (idx - BIAS).astype(np.int16)
    # re-slice per window into gather chunks, appending one safe (>=0)
    # 16-idx column per chunk (the ucode drops a trailing run of negative
    # indices, which both corrupts data and can hang the DMA semaphore)
    safe = np.zeros(16, np.int16)
    pieces = []
    for c in range(CORES):
        for w in range(W):
            K = Ks[w]
            vals = idx16[c, woff[w] * P : (woff[w] + K) * P]
            for toff, ntl in _chunks(K):
                pieces.append(vals[toff * P : (toff + ntl) * P])
                pieces.append(safe)
    percore = sum(len(p) for p in pieces) // CORES
    idx16 = np.concatenate(pieces).reshape(CORES, percore)

    # weights
    W1 = np.asarray(W1, np.float32)
    W1r = W1.reshape(INCH, HEADS, HID)
    wa_s = np.einsum("ihc,hc->ih", W1r, np.asarray(a_src1, np.float32))
    wa_d = np.einsum("ihc,hc->ih", W1r, np.asarray(a_dst1, np.float32))
    wcat = np.concatenate([W1, wa_s, wa_d], axis=1)

    W2 = np.asarray(W2, np.float32)
    w2s = W2 @ np.asarray(a_src2, np.float32)[0]
    w2d = W2 @ np.asarray(a_dst2, np.float32)[0]
    w2cat = np.concatenate([W2, w2s[:, None], w2d[:, None]], axis=1)  # [256, 4]
    w2cat = w2cat.reshape(2, 128, 4).transpose(1, 0, 2)               # [128, 2, 4]

    # x, permuted + transposed: xt[:, g] = x[node]
    bfdt = mybir.dt.np(bf16)
    xt = np.zeros((NROW, INCH), np.float32)
    xt[node_g] = np.asarray(x, np.float32)
    xt = np.ascontiguousarray(xt.T).astype(bfdt)

    in_maps = []
    for c in range(CORES):
        in_maps.append(
            {
                "xt": xt,
                "wcat": wcat.astype(bfdt),
                "w2cat": np.ascontiguousarray(w2cat).astype(bfdt),
                "b1": np.asarray(b1, np.float32).reshape(1, 256),
                "b2": np.asarray(b2, np.float32).reshape(1, 2),
                "idx": _wrap_idx_stream(idx16[c]),
            }
        )

    # output un-permutation: node -> (core, j) -> row core*NPC + j
    out_idx = (rank_of % CORES) * NPC + (rank_of // CORES)
    return tuple(Ks), in_maps, out_idx


class _Runner:
    """Persistent compiled runner: jit once, device-resident inputs, so
    repeated calls time only execution (+ dispatch)."""

    def __init__(self, nc):
        import jax
        from jax.sharding import Mesh, PartitionSpec, NamedSharding
        from jax.experimental.shard_map import shard_map
        from concourse import bass2jax
        import concourse.mybir as mb

        bass2jax.install_neuronx_cc_hook()
        self.jax = jax
        self.nc = nc
        part_name = nc.partition_id_tensor.name if nc.partition_id_tensor else None
        in_names, out_names, out_avals, zero_outs = [], [], [], []
        for alloc in nc.m.functions[0].allocations:
            if not isinstance(alloc, mb.MemoryLocationSet):
                continue
            name = alloc.memorylocations[0].name
            if alloc.kind == "ExternalInput":
                if name != part_name:
                    in_names.append(name)
            elif alloc.kind == "ExternalOutput":
                out_names.append(name)
                shape = tuple(alloc.tensor_shape)
                dtype = mb.dt.np(alloc.dtype)
                out_avals.append(jax.core.ShapedArray(shape, dtype))
                zero_outs.append(np.zeros(shape, dtype))
        self.in_names, self.out_names = in_names, out_names
        self.zero_outs = zero_outs
        n_params, n_outs = len(in_names), len(out_names)
        donate = tuple(range(n_params, n_params + n_outs))

        all_in_names = in_names + out_names + ([part_name] if part_name else [])

        def _body(*args):
            operands = list(args)
            if part_name is not None:
                operands.append(bass2jax.partition_id_tensor())
            outs = bass2jax._bass_exec_p.bind(
                *operands,
                out_avals=tuple(out_avals),
                in_names=tuple(all_in_names),
                out_names=tuple(out_names),
                lowering_input_output_aliases=(),
                sim_require_finite=True,
                sim_require_nnan=True,
                nc=nc,
            )
            return tuple(outs)

        devices = jax.devices()[:CORES]
        self.mesh = Mesh(np.asarray(devices), ("core",))
        self.spec = NamedSharding(self.mesh, PartitionSpec("core"))
        in_specs = (PartitionSpec("core"),) * (n_params + n_outs)
        out_specs = (PartitionSpec("core"),) * n_outs
        self.sharded = jax.jit(
            shard_map(_body, mesh=self.mesh, in_specs=in_specs,
                      out_specs=out_specs, check_rep=False),
            donate_argnums=donate, keep_unused=True,
        )
        self.dev_in = None

    def put_inputs(self, in_maps):
        self.dev_in = [
            self.jax.device_put(
                np.concatenate([np.asarray(m[n]) for m in in_maps], axis=0), self.spec
            )
            for n in self.in_names
        ]

    def execute(self):
        zeros = [
            self.jax.device_put(
                np.zeros((CORES * z.shape[0], *z.shape[1:]), z.dtype), self.spec
            )
            for z in self.zero_outs
        ]
        for z in zeros:
            z.block_until_ready()
        t0 = time.monotonic_ns()
        outs = self.sharded(*self.dev_in, *zeros)
        outs[0].block_until_ready()
        dt = time.monotonic_ns() - t0
        res = [
            {
                name: np.asarray(outs[i]).reshape(CORES, *self.zero_outs[i].shape)[c]
                for i, name in enumerate(self.out_names)
            }
            for c in range(CORES)
        ]
        return res, dt


def run_on_device(in_maps, Ks):
    if Ks not in _cache:
        _cache[Ks] = _Runner(_build(Ks))
    runner = _cache[Ks]
    runner.put_inputs(in_maps)
    res, dt = runner.execute()
    global LAST_EXEC_NS
    LAST_EXEC_NS = dt
    return res


def kernel(x, edge_index, W1, a_src1, a_dst1, b1, W2, a_src2, a_dst2, b2):
    Ks, in_maps, out_idx = _preprocess(
        x, edge_index, W1, a_src1, a_dst1, b1, W2, a_src2, a_dst2, b2
    )
    res = run_on_device(in_maps, Ks)
    out_cat = np.concatenate([res[c]["out"] for c in range(CORES)], axis=0)
    return out_cat[out_idx].astype(np.float32)
